# revision 1
# baseline (speedup 1.0000x reference)
"""Trainium2 Bass kernel: EnhancedSpikingNeuron (LIF, soft reset) forward.

Reference semantics (per element chain (b, d), sequential over t):
    mem = beta * mem + (x[b, t, d] + homeo_i)
    s   = (mem - 1.0 > 0) ? 1.0 : 0.0
    mem = mem - s
Output = spikes [B, T, D] float32.

Implementation notes
--------------------
The recurrence is sequential in t, elementwise-parallel over B*D = 16384
chains.  Sharding: batch-parallel over 8 cores (2 batches/core -> 2048
chains/core = 128 partitions x 16 free elems).

Per-step critical path is exactly 2 fused DVE ops (scalar_tensor_tensor,
out = (in0 op0 scalar) op1 in1), keeping the *pre-reset* membrane u as the
live state (negated between the two ops):
    n_t     = (u_t  >  1.0) - u_t          # = s_t - u_t = -mem_t
    u_{t+1} = (n_t * -beta) + x_{t+1}      # = beta*mem_t + x_{t+1}
This reproduces the reference fp32 rounding exactly:
  fp(s - u) = -fp(u - s), fp((-b)*n) = fp(b*(-n)), and (u - 1 > 0) <=> (u > 1)
  in fp32 (Sterbenz exactness near 1.0).
Spikes are extracted in bulk, one op per K-step block: s = (U_block > 1).

u values for each step land in per-block SBUF tiles U[b] ([128, K*16],
column slice k holds u_{bK+k}); x streams in per block via strided DMA
(64B contiguous chunks per partition), spikes stream out the same way.
"""

import functools
from contextlib import ExitStack

import numpy as np

import concourse.bass as bass
import concourse.bacc as bacc
import concourse.mybir as mybir
import concourse.tile as tile
from concourse.bass_utils import run_bass_kernel_spmd

# Problem geometry (hardcoded per contract).
B, T, D = 16, 2048, 1024
N_CORES = 8
BPC = B // N_CORES          # batches per core = 2
P = 128                     # SBUF partitions
J = 16                      # features per 64B DRAM chunk
PGRP = D // J               # 64 partition-groups per batch
FD = (BPC * D) // P         # 16 free elems per per-step tile
BETA = 0.9
F32 = mybir.dt.float32
Op = mybir.AluOpType


def build_program(T_total: int = T, K: int = 64, h: float = 0.0):
    """Build the single-core Bass/Tile program (same program on all cores)."""
    assert T_total % K == 0
    nblk = T_total // K
    nc = bacc.Bacc("TRN2", target_bir_lowering=False, debug=False)
    x_d = nc.dram_tensor("x", [BPC, T_total, D], F32, kind="ExternalInput")
    s_d = nc.dram_tensor("s", [BPC, T_total, D], F32, kind="ExternalOutput")
    x_ap = x_d.ap()
    s_ap = s_d.ap()

    with tile.TileContext(nc) as tc, ExitStack() as ctx:
        xp = ctx.enter_context(tc.tile_pool(name="xp", bufs=3))
        up = ctx.enter_context(tc.tile_pool(name="up", bufs=3))
        sp = ctx.enter_context(tc.tile_pool(name="sp", bufs=3))
        npool = ctx.enter_context(tc.tile_pool(name="npool", bufs=4))

        X = [None] * nblk
        U = [None] * nblk

        def load(b):
            X[b] = xp.tile([P, K * FD], F32, name=f"x{b}", tag="x")
            for bl in range(BPC):
                src = x_ap[bl, b * K:(b + 1) * K, :].rearrange(
                    "k (p j) -> p k j", p=PGRP, j=J
                )
                dst = X[b][bl * PGRP:(bl + 1) * PGRP, :].rearrange(
                    "p (k j) -> p k j", k=K, j=J
                )
                nc.sync.dma_start(out=dst, in_=src)
            if h != 0.0:
                nc.vector.tensor_scalar(X[b][:, :], X[b][:, :], float(h), None, Op.add)

        load(0)
        U[0] = up.tile([P, K * FD], F32, name="u0", tag="u")
        # u_0 = x_0 (mem starts at 0; beta*0 + x_0 == x_0 exactly)
        nc.vector.tensor_copy(U[0][:, 0:FD], X[0][:, 0:FD])

        for b in range(nblk):
            if b + 1 < nblk:
                load(b + 1)
            for k in range(K):
                t = b * K + k
                if t + 1 >= T_total:
                    break
                ucol = U[b][:, k * FD:(k + 1) * FD]
                n_t = npool.tile([P, FD], F32, name=f"n{t}", tag="n")
                # n = (u > 1) - u
                nc.vector.scalar_tensor_tensor(
                    n_t[:, :], ucol, 1.0, ucol, op0=Op.is_gt, op1=Op.subtract
                )
                if k + 1 < K:
                    unext = U[b][:, (k + 1) * FD:(k + 2) * FD]
                    xcol = X[b][:, (k + 1) * FD:(k + 2) * FD]
                else:
                    U[b + 1] = up.tile([P, K * FD], F32, name=f"u{b + 1}", tag="u")
                    unext = U[b + 1][:, 0:FD]
                    xcol = X[b + 1][:, 0:FD]
                # u' = (n * -beta) + x'
                nc.vector.scalar_tensor_tensor(
                    unext, n_t[:, :], -BETA, xcol, op0=Op.mult, op1=Op.add
                )
            # bulk spike extraction for block b: s = (u > 1)
            s_b = sp.tile([P, K * FD], F32, name=f"s{b}", tag="s")
            nc.vector.tensor_scalar(s_b[:, :], U[b][:, :], 1.0, None, Op.is_gt)
            for bl in range(BPC):
                dst = s_ap[bl, b * K:(b + 1) * K, :].rearrange(
                    "k (p j) -> p k j", p=PGRP, j=J
                )
                src = s_b[bl * PGRP:(bl + 1) * PGRP, :].rearrange(
                    "p (k j) -> p k j", k=K, j=J
                )
                nc.sync.dma_start(out=dst, in_=src)

    nc.compile()
    return nc


@functools.lru_cache(maxsize=2)
def _get_program(h: float, T_total: int = T, K: int = 64):
    return build_program(T_total=T_total, K=K, h=h)


def kernel(x: np.ndarray, homeo_i: np.ndarray) -> np.ndarray:
    x = np.ascontiguousarray(np.asarray(x, dtype=np.float32))
    h = float(np.asarray(homeo_i).reshape(-1)[0])
    assert x.shape == (B, T, D), x.shape
    nc = _get_program(h)
    in_maps = [
        {"x": np.ascontiguousarray(x[c * BPC:(c + 1) * BPC])}
        for c in range(N_CORES)
    ]
    res = run_bass_kernel_spmd(nc, in_maps, list(range(N_CORES)))
    out = np.concatenate([res.results[c]["s"] for c in range(N_CORES)], axis=0)
    return out


# revision 14
# speedup vs baseline: 16886.5798x; 16886.5798x over previous
"""Trainium2 Bass kernel: EnhancedSpikingNeuron (LIF, soft reset) forward.

Reference semantics (per element chain (b, d), sequential over t):
    mem = beta * mem + (x[b, t, d] + homeo_i)
    s   = (mem - 1.0 > 0) ? 1.0 : 0.0
    mem = mem - s
Output = spikes [B, T, D] float32.

Implementation notes
--------------------
The recurrence is sequential in t, elementwise-parallel over B*D = 16384
chains.  Sharding: batch-parallel over 8 cores (2 batches/core -> 2048
chains/core = 128 partitions x 16 free elems).

Per-step critical path is exactly 2 fused DVE ops (scalar_tensor_tensor,
out = (in0 op0 scalar) op1 in1), keeping the *pre-reset* membrane u as the
live state (negated between the two ops):
    n_t     = (u_t  >  1.0) - u_t          # = s_t - u_t = -mem_t
    u_{t+1} = (n_t * -beta) + x_{t+1}      # = beta*mem_t + x_{t+1}
This reproduces the reference fp32 rounding exactly:
  fp(s - u) = -fp(u - s), fp((-b)*n) = fp(b*(-n)), and (u - 1 > 0) <=> (u > 1)
  in fp32 (Sterbenz exactness near 1.0).
Spikes are extracted in bulk, one op per K-step block: s = (U_block > 1).

u values for each step land in per-block SBUF tiles U[b] ([128, K*16],
column slice k holds u_{bK+k}); x streams in per block via strided DMA
(64B contiguous chunks per partition), spikes stream out the same way.
"""

import functools
from contextlib import ExitStack

import numpy as np

import concourse.bass as bass
import concourse.bacc as bacc
import concourse.mybir as mybir
import concourse.tile as tile
from concourse.bass_utils import run_bass_kernel_spmd


def _register_lif_op():
    """Register the fused LIF-step custom DVE op (idempotent, in-process).

    One 4-stage DVE instruction per timestep:
        u' = (u - (u > 1.0)) * beta + x'
    Each stage rounds fp32, reproducing the reference's op-for-op rounding:
    s = H(u-1>0) == (u>1); m = fp(u-s); fp(beta*m); fp(. + x').
    The uop table ships inside the NEFF (dve_table_for_ops), no firmware
    change needed.
    """
    from concourse import dve_ops
    from concourse.dve_spec import Spec, Src0, Src1, C0, C1

    for op in dve_ops.OPS:
        if op.name == "LIF_STEP_ANT":
            return op

    def _ref(in0, in1, s0, s1, imm2):
        s = (in0 > np.float32(s0)).astype(np.float32)
        m = (in0 - s).astype(np.float32)
        return (m * np.float32(s1)).astype(np.float32) + in1

    op = dve_ops.DveOp(
        "LIF_STEP_ANT",
        Spec(body=(Src0 - (Src0 > C0)) * C1 + Src1, reference=_ref),
        subdim=False,
        uops_sha={"v3": "8c1c8b30d434ec6b"},
    )
    dve_ops.OPS.append(op)
    dve_ops._SUB_OPCODE_FOR_NAME[op.name] = (
        dve_ops._CUSTOM_DVE_ROW_BASE + len(dve_ops.OPS) - 1
    )
    dve_ops.CUSTOM_DVE_SPECS[op.name] = op.spec
    return op


LIF_OP = _register_lif_op()

# Problem geometry (hardcoded per contract).
B, T, D = 16, 2048, 1024
N_CORES = 8
BPC = B // N_CORES          # batches per core = 2
P = 128                     # SBUF partitions
J = 16                      # features per 64B DRAM chunk
PGRP = D // J               # 64 partition-groups per batch
FD = (BPC * D) // P         # 16 free elems per per-step tile
BETA = 0.9
F32 = mybir.dt.float32
Op = mybir.AluOpType


def _strip_dve_self_waits(nc):
    """Remove DVE-engine waits on the DVE's own tile-sem lane.

    Tile emits a self-semaphore wait on every DVE op to cover RAW through
    SBUF (write-ack). The DVE executes in order and drains its pipe between
    ops, so same-engine RAW is already safe in hardware; the waits only add
    the ~100ns write-ack round trip per op. Increments are kept so other
    procs' waits on the DVE progress sem stay valid.
    """
    n_strip = 0
    for bb in nc.main_func.blocks:
        for ins in bb.instructions:
            if ins.engine != mybir.EngineType.DVE or ins.sync_info is None:
                continue
            ow = ins.sync_info.on_wait
            if not ow:
                continue
            kept = [w for w in ow
                    if not (w.sync_type == "semaphore"
                            and (w.ant_name or "").startswith("DVE"))]
            if len(kept) != len(ow):
                n_strip += len(ow) - len(kept)
                ins.sync_info.on_wait = kept
    return n_strip


def build_program(T_total: int = T, K: int = 64, h: float = 0.0, reps: int = 1,
                  elide_dve_self_waits: bool = False,
                  extract_on_pool: bool = False,
                  interleave: int = 1,
                  skip_extract: bool = False):
    """Build the single-core Bass/Tile program (same program on all cores).

    reps > 1 wraps the whole computation in a hardware loop (for timing
    measurements via wall-clock slope; the computation is idempotent).
    """
    assert T_total % K == 0
    nblk = T_total // K
    nc = bacc.Bacc("TRN2", target_bir_lowering=False, debug=False)
    x_d = nc.dram_tensor("x", [BPC, T_total, D], F32, kind="ExternalInput")
    s_d = nc.dram_tensor("s", [BPC, T_total, D], F32, kind="ExternalOutput")
    x_ap = x_d.ap()
    s_ap = s_d.ap()

    with tile.TileContext(nc) as tc, ExitStack() as ctx:
        if reps > 1:
            ctx.enter_context(tc.For_i(0, reps, 1))
        xp = ctx.enter_context(tc.tile_pool(name="xp", bufs=3))
        up = ctx.enter_context(tc.tile_pool(name="up", bufs=3))
        sp = ctx.enter_context(tc.tile_pool(name="sp", bufs=3))

        X = [None] * nblk
        U = [None] * nblk

        def load(b):
            X[b] = xp.tile([P, K * FD], F32, name=f"x{b}", tag="x")
            for bl in range(BPC):
                src = x_ap[bl, b * K:(b + 1) * K, :].rearrange(
                    "k (p j) -> p k j", p=PGRP, j=J
                )
                dst = X[b][bl * PGRP:(bl + 1) * PGRP, :].rearrange(
                    "p (k j) -> p k j", k=K, j=J
                )
                nc.sync.dma_start(out=dst, in_=src)
            if h != 0.0:
                nc.vector.tensor_scalar(X[b][:, :], X[b][:, :], float(h), None, Op.add)

        load(0)
        U[0] = up.tile([P, K * FD], F32, name="u0", tag="u")
        # u_0 = x_0 (mem starts at 0; beta*0 + x_0 == x_0 exactly).
        # Split per sub-chain so the first LIF op is `interleave` ops away
        # from the copy that produced its input.
        for i in range(interleave):
            lo, hi = i * (FD // interleave), (i + 1) * (FD // interleave)
            nc.vector.tensor_copy(U[0][:, lo:hi], X[0][:, lo:hi])

        for b in range(nblk):
            if b + 1 < nblk:
                load(b + 1)
            for k in range(K):
                t = b * K + k
                if t + 1 >= T_total:
                    break
                if k + 1 == K:
                    U[b + 1] = up.tile([P, K * FD], F32, name=f"u{b + 1}", tag="u")
                # interleave>1 splits the FD columns into independent
                # sub-chains so consecutive DVE ops have no data dependency
                # (RAW distance = interleave ops).
                sub = FD // interleave
                for i in range(interleave):
                    lo, hi = i * sub, (i + 1) * sub
                    ucol = U[b][:, k * FD + lo:k * FD + hi]
                    if k + 1 < K:
                        unext = U[b][:, (k + 1) * FD + lo:(k + 1) * FD + hi]
                        xcol = X[b][:, (k + 1) * FD + lo:(k + 1) * FD + hi]
                    else:
                        unext = U[b + 1][:, lo:hi]
                        xcol = X[b + 1][:, lo:hi]
                    # u' = (u - (u > 1)) * beta + x'  (one fused DVE op)
                    nc.vector._custom_dve(
                        LIF_OP, out=unext, in0=ucol, in1=xcol, s0=1.0, s1=BETA
                    )
            if skip_extract:  # timing-decomposition only: chain + in-DMA
                continue
            # bulk spike extraction for block b: s = (u > 1)
            s_b = sp.tile([P, K * FD], F32, name=f"s{b}", tag="s")
            eng = nc.gpsimd if extract_on_pool else nc.vector
            eng.tensor_scalar(s_b[:, :], U[b][:, :], 1.0, None, Op.is_gt)
            for bl in range(BPC):
                dst = s_ap[bl, b * K:(b + 1) * K, :].rearrange(
                    "k (p j) -> p k j", p=PGRP, j=J
                )
                src = s_b[bl * PGRP:(bl + 1) * PGRP, :].rearrange(
                    "p (k j) -> p k j", k=K, j=J
                )
                nc.sync.dma_start(out=dst, in_=src)

    if elide_dve_self_waits:
        _strip_dve_self_waits(nc)
    nc.compile()
    return nc


@functools.lru_cache(maxsize=2)
def _get_program(h: float, T_total: int = T, K: int = 128):
    return build_program(T_total=T_total, K=K, h=h)


def kernel(x: np.ndarray, homeo_i: np.ndarray) -> np.ndarray:
    x = np.ascontiguousarray(np.asarray(x, dtype=np.float32))
    h = float(np.asarray(homeo_i).reshape(-1)[0])
    assert x.shape == (B, T, D), x.shape
    nc = _get_program(h)
    in_maps = [
        {"x": np.ascontiguousarray(x[c * BPC:(c + 1) * BPC])}
        for c in range(N_CORES)
    ]
    res = run_bass_kernel_spmd(nc, in_maps, list(range(N_CORES)))
    out = np.concatenate([res.results[c]["s"] for c in range(N_CORES)], axis=0)
    return out
